# revision 1
# baseline (speedup 1.0000x reference)
"""Multi-head attention with LoRA adapters on 8 Trainium2 NeuronCores.

Problem: x[4,2048,768] -> LoRA-linear QKV -> 12-head attention -> LoRA-linear out proj.

Math notes:
  - LoRA is folded into the base weights on the host:  x@W.T + b + (x@A.T)@B.T
    == x@(W + B@A).T + b  (exact up to fp rounding).
  - The value bias bv is folded into the output bias using softmax(row)@1 == 1:
    (attn@(v + bv)) @ Wo.T + bo == (attn@v)@Wo.T + (bo + Wo@bv).
  - Softmax without max-subtraction (scores are O(+-10), safe in fp32); the row
    sum rides as a ones column appended to v (M=65 PV matmuls), and the division
    is applied to the tiny PV output, not the attention matrix.

Sharding: core = 2*b + g for batch b in 0..3, head-group g in 0..1 (6 heads each).
Each core computes its 6 heads' attention and a row-sharded partial output
projection; the host sums the two bf16 partials per batch.

Schedule (ACT exp is the per-step rate limit; the PE must never idle):
  - Attention step s of a (head, 1024-col j-chunk) unit: scores psB[128,1024]
    (2 PE matmuls) -> exp on ACT (1038ns) -> PV into psC[65,1024] (2 PE
    matmuls).  PE attention work is 853ns/step, so the PE queue is padded with
    paced "filler" matmuls (other projections) to stay busy (and DVFS-ramped).
  - PV is emitted TWO steps behind its score so the score feeding exp_{s+1}
    always completes before exp_s retires (ACT never gaps on psB reuse).
  - psC is drained by two fast PSUM->SBUF copies; the softmax division happens
    from SBUF afterwards (DVE reciprocal row64->part0, Pool partition_broadcast,
    DVE multiply writing straight into outT's partition half), so psC needs
    only one 2-bank buffer.
  - PSUM budget (8 banks): psB 2x[128,1024] (4) + psC [65,1024] (2) +
    filler psp 2x[128,512] (2).
  - Prologue: per-kk DMA interleave of wk/wq with xT blocks; k(cb0) jt0-3 and
    q(cb0) jt0-1 accumulate kk-OUTER in the psB/psC banks so the PE starts as
    soon as the first blocks land; q jt2-3, v, later-cb projections and the
    partial output projections all ride the filler stream (demand-forced via
    markers where an attention matmul will consume their results).
  - Output projection partials: sOutA = outT(cb0)@wo(cb0)+bias and
    sOutB = sOutA + outT(cb1)@wo(cb1), both staged bf16 during cb2's
    attention; the epilogue adds the cb2 term.  cb2 runs its units j-outer so
    the j0 epilogue tiles (t 0:1024) stream out across the last two units;
    only the final j1 drain gates the last 8 tiles.  Partials stream out bf16
    and the host sums them in fp32.

TimelineSim (cost model): 251.9us/core vs 305.3us for the previous kernel;
PE busy ~225us = the bf16 PE-column floor for this decomposition (540,672
matmul columns/core at 2.4GHz).  fp8 DoubleRow was evaluated and rejected:
e4m3's ~3.6% element error does not average out through softmax-weighted
sums, putting the estimated max error at the 2e-2 gate.
"""

import sys

sys.path.insert(0, "/opt/trn_rl_repo")

import numpy as np

DIM, HEADS, R = 768, 12, 8
B, T = 4, 2048
HD = DIM // HEADS          # 64 head dim
NCORES = 8
HG = HEADS // 2            # 6 heads per core
CS = HG * HD               # 384 local channels per core
SCALE = HD ** -0.5

_PROGRAM_CACHE = {}


def _bf16(a):
    import ml_dtypes
    return np.ascontiguousarray(a).astype(ml_dtypes.bfloat16)


class _Filler:
    """Paced stream of independent PE work (plus trailing DVE ops) that is
    interleaved into the attention steps to absorb the ACT-vs-PE deficit.
    Markers allow demand-driven forcing (pace_until) for items whose results
    an upcoming attention matmul depends on."""

    def __init__(self):
        self.items = []          # (cols, fn)
        self.total = 0
        self.pos = 0
        self.done = 0

    def add(self, cols, fn):
        self.items.append((cols, fn))
        self.total += cols

    def mark(self):
        return len(self.items) - 1

    def pace(self, frac):
        target = self.total * min(frac, 1.0)
        while self.pos < len(self.items) and self.done < target:
            cols, fn = self.items[self.pos]
            fn()
            self.done += cols
            self.pos += 1

    def pace_until(self, idx):
        while self.pos <= idx:
            cols, fn = self.items[self.pos]
            fn()
            self.done += cols
            self.pos += 1

    def flush(self):
        self.pace(2.0)


def _build_program():
    import concourse.bass as bass
    import concourse.mybir as mybir
    import concourse.tile as tile
    from concourse import bacc

    f32 = mybir.dt.float32
    bf16 = mybir.dt.bfloat16

    nc = bacc.Bacc("TRN2", target_bir_lowering=False, debug=False,
                   num_devices=NCORES)

    xT = nc.dram_tensor("xT", [DIM, T], bf16, kind="ExternalInput")
    wq_t = nc.dram_tensor("wq_t", [DIM, CS], bf16, kind="ExternalInput")
    wk_t = nc.dram_tensor("wk_t", [DIM, CS], bf16, kind="ExternalInput")
    wv_t = nc.dram_tensor("wv_t", [DIM, CS], bf16, kind="ExternalInput")
    wo_t = nc.dram_tensor("wo_t", [CS, DIM], bf16, kind="ExternalInput")
    bq_s = nc.dram_tensor("bq_s", [CS], f32, kind="ExternalInput")
    bk_s = nc.dram_tensor("bk_s", [CS], f32, kind="ExternalInput")
    bo_s = nc.dram_tensor("bo_s", [DIM], f32, kind="ExternalInput")
    ident_d = nc.dram_tensor("ident", [128, 128], bf16, kind="ExternalInput")
    out_p = nc.dram_tensor("out_p", [T, DIM], bf16, kind="ExternalOutput")

    KB = DIM // 128      # 6 k-blocks of the input dim
    CB = CS // 128       # 3 channel blocks (head pairs)
    TB = T // 128        # 16 s tiles
    NJ = T // 1024       # 2 j chunks per head
    VW = HD + 1          # 65: v plus ones column (ones at index 64)

    with tile.TileContext(nc) as tc:
        with (
            tc.tile_pool(name="weights", bufs=1) as wpool,
            tc.tile_pool(name="psB", bufs=2, space="PSUM") as psB_pool,
            tc.tile_pool(name="psC", bufs=1, space="PSUM") as psC_pool,
            tc.tile_pool(name="psp", bufs=2, space="PSUM") as psp_pool,
            tc.tile_pool(name="epool", bufs=5) as e_pool,
            tc.tile_pool(name="stage", bufs=3) as st_pool,
            tc.tile_pool(name="npool", bufs=3) as n_pool,
            tc.tile_pool(name="opool", bufs=8) as o_pool,
        ):
            # ---- input loads: interleave wk/wq blocks with xT blocks so the
            # kk-outer prologue can start as soon as block 0 lands ----
            w_kt = wpool.tile([128, KB, CS], bf16)
            w_qt = wpool.tile([128, KB, CS], bf16)
            xT_sb = wpool.tile([128, KB, T], bf16)
            wk_view = wk_t.ap().rearrange("(k p) m -> p k m", p=128)
            wq_view = wq_t.ap().rearrange("(k p) m -> p k m", p=128)
            xT_view = xT.ap().rearrange("(k p) t -> p k t", p=128)
            bq_sb = wpool.tile([128, CB], f32)
            bk_sb = wpool.tile([128, CB], f32)
            for kk in range(KB):
                if kk % 2 == 0:
                    nc.scalar.dma_start(out=w_kt[:, kk:kk + 2, :],
                                        in_=wk_view[:, kk:kk + 2, :])
                    nc.scalar.dma_start(out=w_qt[:, kk:kk + 2, :],
                                        in_=wq_view[:, kk:kk + 2, :])
                if kk in (0, 4, 5):
                    # halves: kk0 so the first pass starts one transfer early,
                    # kk4/kk5 so the finishing groups start per half
                    nc.sync.dma_start(out=xT_sb[:, kk, 0:1024],
                                      in_=xT_view[:, kk, 0:1024])
                    nc.sync.dma_start(out=xT_sb[:, kk, 1024:2048],
                                      in_=xT_view[:, kk, 1024:2048])
                else:
                    nc.sync.dma_start(out=xT_sb[:, kk, :], in_=xT_view[:, kk, :])
                if kk == 2:
                    nc.scalar.dma_start(
                        out=bk_sb, in_=bk_s.ap().rearrange("(k p) -> p k", p=128))
                    nc.scalar.dma_start(
                        out=bq_sb, in_=bq_s.ap().rearrange("(k p) -> p k", p=128))
            w_vt = wpool.tile([128, KB, CS], bf16)
            nc.sync.dma_start(out=w_vt, in_=wv_t.ap().rearrange("(k p) m -> p k m", p=128))
            w_ot = wpool.tile([128, CB, DIM], bf16)
            nc.sync.dma_start(out=w_ot, in_=wo_t.ap().rearrange("(k p) m -> p k m", p=128))
            bo_row = wpool.tile([1, DIM], f32)
            nc.sync.dma_start(out=bo_row, in_=bo_s.ap().rearrange("(o d) -> o d", o=1))
            bo_sb = wpool.tile([128, DIM], f32)
            nc.gpsimd.partition_broadcast(bo_sb, bo_row)
            ident = wpool.tile([128, 128], bf16)
            nc.scalar.dma_start(out=ident, in_=ident_d.ap())

            # preload the exp table off the critical path
            scr = wpool.tile([1, CB], f32)
            nc.scalar.activation(scr, bq_sb[0:1, :], mybir.ActivationFunctionType.Exp)

            # ---- persistent activations ----
            qT_sb = wpool.tile([128, CB, T], bf16)
            kT_sb = wpool.tile([128, CB, T], bf16)
            v_sb = wpool.tile([128, TB, HG * VW], bf16)
            outT_sb = wpool.tile([128, CB, T], bf16)
            sOutA = wpool.tile([128, TB, DIM], bf16)
            sOutB = wpool.tile([128, TB, DIM], bf16)

            # ones columns of v_aug (one strided memset covers all 16x6 columns)
            ones_ap = bass.AP(
                tensor=v_sb.tensor, offset=v_sb.offset + HD,
                ap=[v_sb.ap[0], [HG * VW, TB], [VW, HG]],
            )
            nc.vector.memset(ones_ap.bitcast(mybir.dt.uint16), 0x3F80)

            # ---- prologue: k(cb0) jt0-3 and q(cb0) jt0-1, kk-OUTER so the PE
            # tracks the xT DMA stream; finish kk4/5 per group + bias adds ----
            kAcc = [psB_pool.tile([128, 1024], f32, tag="psB", name="kAcc")
                    for _ in range(2)]
            qAcc = psC_pool.tile([128, 1024], f32, tag="psC", name="qAcc")

            def pro_mm(acc, w, jt, kk, start, stop):
                nc.tensor.matmul(
                    acc[:, (jt % 2) * 512:(jt % 2) * 512 + 512],
                    w[:, kk, 0:128],
                    xT_sb[:, kk, jt * 512:(jt + 1) * 512],
                    start=start, stop=stop,
                )

            for kk in range(4):
                for jt in range(4):
                    pro_mm(kAcc[jt // 2], w_kt, jt, kk, kk == 0, False)
                for jt in range(2):
                    pro_mm(qAcc, w_qt, jt, kk, kk == 0, False)
            # finish half-a consumers (k jt0/1, q jt0/1) first so the PE has
            # work while the second halves of xT4/xT5 are still in flight —
            # keeps the DVFS busy-streak unbroken into the attention start
            pro_groups = [(kAcc[jt // 2], w_kt, kT_sb, bk_sb, jt) for jt in range(2)]
            pro_groups += [(qAcc, w_qt, qT_sb, bq_sb, jt) for jt in range(2)]
            pro_groups += [(kAcc[jt // 2], w_kt, kT_sb, bk_sb, jt) for jt in range(2, 4)]
            for acc, w, dst, bias, jt in pro_groups:
                pro_mm(acc, w, jt, 4, False, False)
                pro_mm(acc, w, jt, 5, False, True)
                nc.vector.tensor_scalar_add(
                    dst[:, 0, jt * 512:(jt + 1) * 512],
                    acc[:, (jt % 2) * 512:(jt % 2) * 512 + 512],
                    bias[:, 0:1],
                )

            # ---- filler work generators ----

            def kq_quanta(filler, w, dst, bias, cb, jts=range(4)):
                last = None
                for jt in jts:
                    state = {}
                    for kk in range(KB):
                        def fn(jt=jt, kk=kk, state=state):
                            if kk == 0:
                                state["t"] = psp_pool.tile(
                                    [128, 512], f32, tag="psp", name="pspq")
                            cols = slice(jt * 512, (jt + 1) * 512)
                            nc.tensor.matmul(
                                state["t"], w[:, kk, cb * 128:(cb + 1) * 128],
                                xT_sb[:, kk, cols],
                                start=(kk == 0), stop=(kk == KB - 1),
                            )
                            if kk == KB - 1:
                                nc.vector.tensor_scalar_add(
                                    dst[:, cb, cols], state["t"], bias[:, cb:cb + 1])
                        filler.add(512, fn)
                    last = filler.mark()
                return last

            def emit_v_scatter(psp, st0, nst, c0, nch):
                for i in range(nst):
                    src = psp[:, i * nch:(i + 1) * nch].rearrange(
                        "p (h c) -> p h c", c=HD)
                    dst = bass.AP(
                        tensor=v_sb.tensor,
                        offset=v_sb.offset + (st0 + i) * (HG * VW) + (c0 // HD) * VW,
                        ap=[v_sb.ap[0], [VW, nch // HD], [1, HD]],
                    )
                    nc.vector.tensor_copy(dst, src)

            def v_quanta(filler, c0, nch, nst, vmark, cbs):
                # v-projection: groups of nst s-tiles x nch channels
                for st0 in range(0, TB, nst):
                    state = {}
                    nmm = nst * KB
                    for m in range(nmm):
                        def fn(m=m, st0=st0, state=state):
                            if m == 0:
                                state["t"] = psp_pool.tile(
                                    [128, nst * nch], f32, tag="psp", name="pspv")
                            i, kk = divmod(m, KB)
                            nc.tensor.matmul(
                                state["t"][:, i * nch:(i + 1) * nch],
                                xT_sb[:, kk, (st0 + i) * 128:(st0 + i + 1) * 128],
                                w_vt[:, kk, c0:c0 + nch],
                                start=(kk == 0), stop=(kk == KB - 1),
                            )
                            if m == nmm - 1:
                                emit_v_scatter(state["t"], st0, nst, c0, nch)
                        filler.add(nch, fn)
                    idx = filler.mark()
                    for cb in cbs:
                        for st in range(st0, st0 + nst):
                            vmark[(cb, st)] = (filler, idx)

            def oproj_quanta(filler, cb, dst, addend):
                # partial output projection for channel block cb (bf16 staging):
                #   dst[mt] = outT[cb]^T @ wo[cb] + addend
                for mt in range(TB):
                    for half in range(2):
                        cols = slice(half * 384, half * 384 + 384)
                        def fn(mt=mt, cols=cols):
                            psp = psp_pool.tile([128, 384], f32, tag="psp",
                                                name="pspo")
                            nc.tensor.matmul(
                                psp, outT_sb[:, cb, mt * 128:(mt + 1) * 128],
                                w_ot[:, cb, cols], start=True, stop=True,
                            )
                            if addend is sOutA:
                                nc.vector.tensor_add(
                                    dst[:, mt, cols], psp, addend[:, mt, cols])
                            else:
                                nc.vector.tensor_add(dst[:, mt, cols], psp,
                                                     addend[:, cols])
                        filler.add(384, fn)

            def emit_pv(item):
                s, e, psC, h = item
                lhs_v = v_sb[:, s, h * VW:(h + 1) * VW]
                for half in range(2):
                    cols = slice(half * 512, half * 512 + 512)
                    nc.tensor.matmul(
                        psC[:, cols], lhs_v, e[:, cols],
                        start=(s == 0), stop=(s == TB - 1),
                    )

            def emit_drain(item, post_half=None, direct=False):
                # psC -> SBUF stage (frees the PSUM banks bank-by-bank), then:
                # recip of row 64 into partition 0, Pool broadcast to 64 rows,
                # multiply into outT's partition half for this head.
                # direct=True (final unit only): skip the staging copies and
                # normalize straight out of PSUM — nothing needs the banks.
                psC, h, j = item
                hb = h % 2
                cb = h // 2
                if direct:
                    stage = psC
                else:
                    stage = st_pool.tile([VW, 1024], f32, tag="st")
                    nc.vector.tensor_copy(stage[:, 0:512], psC[:, 0:512])
                    nc.vector.tensor_copy(stage[:, 512:1024], psC[:, 512:1024])
                nr0 = n_pool.tile([1, 1024], f32, tag="nr0")
                nrb = n_pool.tile([HD, 1024], f32, tag="nrb")
                prow = slice(64 * hb, 64 * hb + 64)
                # halves pipeline: recip (DVE) -> broadcast (Pool) -> mul (DVE)
                for ha in range(2):
                    hc = slice(ha * 512, ha * 512 + 512)
                    nc.vector.reciprocal(nr0[:, hc], stage[HD:VW, hc])
                    nc.gpsimd.partition_broadcast(nrb[:, hc], nr0[:, hc])
                for ha in range(2):
                    hc = slice(ha * 512, ha * 512 + 512)
                    nc.vector.tensor_mul(
                        outT_sb[prow, cb, j * 1024 + ha * 512:
                                j * 1024 + ha * 512 + 512],
                        stage[0:HD, hc], nrb[:, hc],
                    )
                    if post_half is not None:
                        post_half(ha)

            epi_osb = {}

            def emit_epi(mt, half):
                # epilogue: add the cb2 projection term to the staged partials
                # and stream the bf16 partial row out (one DMA per mt)
                cols = slice(half * 384, half * 384 + 384)
                psp = psp_pool.tile([128, 384], f32, tag="psp", name="pspe")
                nc.tensor.matmul(
                    psp, outT_sb[:, 2, mt * 128:(mt + 1) * 128],
                    w_ot[:, 2, cols], start=True, stop=True,
                )
                if half == 0:
                    epi_osb[mt] = o_pool.tile([128, DIM], bf16, tag="osb", name="osb")
                osb = epi_osb[mt]
                nc.vector.tensor_add(osb[:, cols], psp, sOutB[:, mt, cols])
                if half == 1:
                    nc.sync.dma_start(
                        out=out_p.ap()[mt * 128:(mt + 1) * 128, :], in_=osb)
                    del epi_osb[mt]

            # ---- filler streams per channel block ----
            vmark, qmark = {}, {}
            f0 = _Filler()
            v_quanta(f0, 0, 128, 4, vmark, [0])        # v(cb0), demand-forced
            qmark[(0, 1)] = (f0, kq_quanta(f0, w_qt, qT_sb, bq_sb, 0, jts=(2, 3)))
            kq_quanta(f0, w_kt, kT_sb, bk_sb, 1)
            kq_quanta(f0, w_qt, qT_sb, bq_sb, 1)
            v_quanta(f0, 128, 128, 4, vmark, [1])      # v(cb1)
            f1 = _Filler()
            kq_quanta(f1, w_kt, kT_sb, bk_sb, 2)
            kq_quanta(f1, w_qt, qT_sb, bq_sb, 2)
            v_quanta(f1, 256, 128, 4, vmark, [2])      # v(cb2)
            f2 = _Filler()
            oproj_quanta(f2, 0, sOutA, bo_sb)
            oproj_quanta(f2, 1, sOutB, sOutA)
            fillers = [f0, f1, f2]

            # ---- attention: 12 units of (head, j) x 16 steps, ACT-clocked ----
            pend = []       # up to 2 (s, e, psC, h) awaiting their PV matmuls
            dpend = []      # (psC, h, j) awaiting drain (after the last PV)
            epi_done = 0

            def pop_pv():
                it = pend.pop(0)
                m = vmark.get((it[3] // 2, it[0]))
                if m is not None:
                    m[0].pace_until(m[1])
                emit_pv(it)
                if it[0] == TB - 1:
                    emit_drain(dpend.pop(0))

            for cb in range(CB):
                filler = fillers[cb]
                step = 0
                # cb2 runs j-outer so the j0 epilogue tiles spread across the
                # last TWO units instead of saturating DVE in the final one
                if cb == 2:
                    unit_list = [(hb, j) for j in range(NJ) for hb in range(2)]
                else:
                    unit_list = [(hb, j) for hb in range(2) for j in range(NJ)]
                for ui, (hb, j) in enumerate(unit_list):
                    if True:
                        h = 2 * cb + hb
                        prow = slice(64 * hb, 64 * hb + 64)
                        last_unit = (cb == 2 and ui >= 2)
                        m = qmark.get((cb, j))
                        if m is not None:
                            m[0].pace_until(m[1])
                        psC = psC_pool.tile([VW, 1024], f32, tag="psC")
                        for s in range(TB):
                            psB = psB_pool.tile([128, 1024], f32, tag="psB")
                            lhs_k = kT_sb[prow, cb, s * 128:(s + 1) * 128]
                            for half in range(2):
                                cols = slice(half * 512, half * 512 + 512)
                                tcols = slice(j * 1024 + half * 512,
                                              j * 1024 + half * 512 + 512)
                                nc.tensor.matmul(
                                    psB[:, cols], lhs_k, qT_sb[prow, cb, tcols],
                                    start=True, stop=True,
                                )
                            e = e_pool.tile([128, 1024], bf16, tag="e")
                            nc.scalar.activation(
                                e, psB, mybir.ActivationFunctionType.Exp,
                                scale=SCALE,
                            )
                            pend.append((s, e, psC, h))
                            if len(pend) > 2:
                                pop_pv()
                            first_unit = (cb == 0 and hb == 0 and j == 0)
                            if s >= 2 or first_unit:
                                if cb == 0:
                                    filler.pace((step + 1) / 62.0)
                                elif step >= 2:
                                    denom = 60.0 if cb == 1 else 52.0
                                    filler.pace((step - 1) / denom)
                            if last_unit:
                                es = (ui - 2) * TB + s
                                if es >= 3:
                                    want = min(16, (es - 2) * 16 // 26)
                                    while epi_done < want:
                                        emit_epi(epi_done // 2, epi_done % 2)
                                        epi_done += 1
                            step += 1
                        dpend.append((psC, h, j))
                filler.flush()

            def emit_epi_full(mt):
                # cb2 term plus an identity-matmul fold of the staged partials,
                # then an ACT copy out of PSUM (DVE stays out of the tail)
                pool = psB_pool if mt % 2 == 0 else psC_pool
                psD = pool.tile([128, DIM], f32, tag=pool is psB_pool and "psB" or "psC",
                                name="psD")
                for cols in (slice(0, 512), slice(512, DIM)):
                    nc.tensor.matmul(
                        psD[:, cols], outT_sb[:, 2, mt * 128:(mt + 1) * 128],
                        w_ot[:, 2, cols], start=True, stop=False,
                    )
                    nc.tensor.matmul(
                        psD[:, cols], ident, sOutB[:, mt, cols],
                        start=False, stop=True,
                    )
                osb = o_pool.tile([128, DIM], bf16, tag="osb", name="osbf")
                nc.scalar.copy(osb, psD)
                nc.sync.dma_start(
                    out=out_p.ap()[mt * 128:(mt + 1) * 128, :], in_=osb)

            def final_post_half(ha):
                # mt 8-11 depend only on the low half of the final drain
                for mt in range(8 + 4 * ha, 12 + 4 * ha):
                    emit_epi_full(mt)

            while pend:
                it = pend.pop(0)
                emit_pv(it)
                if it[0] == TB - 1:
                    while epi_done < 16:
                        emit_epi(epi_done // 2, epi_done % 2)
                        epi_done += 1
                    emit_drain(dpend.pop(0), post_half=final_post_half, direct=True)

    nc.compile()
    return nc


def _get_program():
    if "nc" not in _PROGRAM_CACHE:
        _PROGRAM_CACHE["nc"] = _build_program()
    return _PROGRAM_CACHE["nc"]


def _prep_in_maps(inputs):
    f = np.float32
    # harness may pass JAX arrays; coerce so the float64 LoRA fold is real
    inputs = {k: np.asarray(v) for k, v in inputs.items()}

    def eff(w, a, bl):
        return (w.astype(np.float64) + bl.astype(np.float64) @ a.astype(np.float64)).astype(f)

    wq = eff(inputs["wq"], inputs["laq"], inputs["lbq"])
    wk = eff(inputs["wk"], inputs["lak"], inputs["lbk"])
    wv = eff(inputs["wv"], inputs["lav"], inputs["lbv"])
    wo = eff(inputs["wo"], inputs["lao"], inputs["lbo"])
    x = np.asarray(inputs["x"], dtype=f)
    bq, bk, bv, bo = (np.asarray(inputs[k], dtype=f) for k in ("bq", "bk", "bv", "bo"))

    in_maps = []
    for core in range(NCORES):
        b, g = core // 2, core % 2
        cols = slice(g * CS, (g + 1) * CS)
        bo_core = wo[:, cols].astype(np.float64) @ bv[cols].astype(np.float64)
        if g == 0:
            bo_core = bo_core + bo
        in_maps.append({
            "xT": _bf16(x[b].T),
            "wq_t": _bf16(wq[cols, :].T),
            "wk_t": _bf16(wk[cols, :].T),
            "wv_t": _bf16(wv[cols, :].T),
            "wo_t": _bf16(wo[:, cols].T),
            "bq_s": np.ascontiguousarray(bq[cols]),
            "bk_s": np.ascontiguousarray(bk[cols]),
            "bo_s": bo_core.astype(f),
            "ident": _bf16(np.eye(128, dtype=f)),
        })
    return in_maps


def kernel(**inputs):
    from concourse.bass_utils import run_bass_kernel_spmd

    nc = _get_program()
    in_maps = _prep_in_maps(inputs)
    res = run_bass_kernel_spmd(nc, in_maps, core_ids=list(range(NCORES)))
    out = np.empty((B, T, DIM), dtype=np.float32)
    for b in range(B):
        out[b] = (res.results[2 * b]["out_p"].astype(np.float32)
                  + res.results[2 * b + 1]["out_p"].astype(np.float32))
    return out

